# revision 1
# baseline (speedup 1.0000x reference)
"""DISCO (discrete-continuous) S2 conv encoder for Trainium2, 8-core SPMD.

Math (per output latitude h):
    z[k, c, w] = sum_n psi[k, h, n] * x[c, hi[h,n], (wi[h,n] + 2w) mod Win]
    y[o, h, w] = sum_{c,k} weight[o, c, k] * z[k, c, w]

Strategy:
  * Shard output latitudes (Hout) across the 8 cores; weight/psi tables
    replicated; each core receives the full (re-laid-out) x.
  * x is quantized host-side to float8_e3m4 (prescale XSCALE, clip to
    +-15.5); the 1/XSCALE dequant is folded into the f16 psi table.
    Mixed-dtype matmul (f16 lhsT x fp8 rhs) is legal on TRN2 and halves
    the gather DMA traffic, the dominant cost. Final rel err ~1.3e-2.
  * Host pre-lays x out as parity-split (even/odd longitude), cyclically
    padded, channel-minor rows:  xpad[par, r, w', c], w' in [0, 2*Wout-1).
    Then the (h, n) support slice {x[c, hi, wi + 2w] : w in [w0, w0+WB),
    all c} is ONE contiguous fp8 block, so the device gathers 128 of them
    (one per neighbor n) with a single indirect DMA per (h, w-block).
  * matmul1 contracts n (=128, the partition dim) with the per-h psi basis
    vT [128, K]; 4 latitudes run concurrently in separate 32-column
    tile_position groups of the PE array.
  * z is evacuated from PSUM (f32->f16, alternating DVE/ACT engines) and
    staged in NHALF w-spans; each completed span is transposed via
    SBUF->SBUF DMA (>=512B runs) into (k,c)-partition span tiles.
  * matmul2 consumes each span immediately (double-buffered per
    latitude), contracting (c,k) in NG accumulating chunks against the
    packed weight w2 [K*CG, NG, Cout]; y evacuated f16, host upcasts.
"""

import math
from contextlib import ExitStack
from dataclasses import dataclass

import numpy as np


# ---------------------------------------------------------------- dims

@dataclass(frozen=True)
class Dims:
    Cin: int = 73
    Hin: int = 721
    Win: int = 1440
    Cout: int = 256
    K: int = 9
    Hout: int = 361
    N: int = 128
    pscale: int = 2
    ncores: int = 8
    GH: int = 4     # latitudes processed concurrently (PE col-tile groups)
    WB: int = 90    # w-block size for the gather/matmul1 pipeline
    XB: int = 2     # gather buffer depth per latitude tag
    NHALF: int = 2  # z4 is staged in NHALF w-spans, transposed per span
    XSCALE: float = 2.0  # x prescale for float8e3 quantization
    # "full" | "fakegather" | pipeline truncations for perf ablation:
    # "gather" (gathers+y only), "mm1", "evac", "tr" (everything but mm2)
    mode: str = "full"
    REP: int = 1    # repeat the whole body (device-side timing ablation)

    @property
    def stage(self):
        order = {"gather": 1, "mm1": 2, "evac": 3, "tr": 4,
                 "full": 5, "fakegather": 5}
        return order[self.mode]

    @property
    def Wout(self):
        return self.Win // self.pscale

    @property
    def WPAD(self):
        return 2 * self.Wout - 1

    @property
    def TOT(self):
        return 2 * self.Hin * self.WPAD * self.Cin

    @property
    def NWB(self):
        assert self.Wout % self.WB == 0
        return self.Wout // self.WB

    @property
    def WBH(self):  # w-blocks per z4 half-span
        assert self.NWB % self.NHALF == 0
        return self.NWB // self.NHALF

    @property
    def HWB(self):  # w width of a z4 span
        return self.WBH * self.WB

    @property
    def HB(self):  # padded per-core latitude slots
        per = math.ceil(self.Hout / self.ncores)
        return math.ceil(per / self.GH) * self.GH

    @property
    def NGRP(self):
        return self.HB // self.GH

    @property
    def KP(self):  # psi k-dim padded to a full PE column-tile group
        return 32

    @property
    def CG(self):  # channels per (c,k) partition group of z'
        return min(128 // self.K, self.Cin)

    @property
    def P2(self):
        return self.K * self.CG

    @property
    def NG(self):
        return math.ceil(self.Cin / self.CG)

    @property
    def CREM(self):  # channels in last group
        return self.Cin - self.CG * (self.NG - 1)

    @property
    def CC(self):  # channel chunk for matmul1 psum (<=512 f32 per bank)
        return max(1, min(512 // self.WB, self.Cin))

    @property
    def NCC(self):
        return math.ceil(self.Cin / self.CC)

    @property
    def OH(self):  # output-channel halves
        return math.ceil(self.Cout / 128)

    @property
    def OHW(self):
        return min(self.Cout, 128)

    @property
    def NWH(self):  # w-halves for matmul2 (<=512 f32 psum)
        return math.ceil(self.Wout / 512)

    @property
    def WHS(self):
        assert self.Wout % self.NWH == 0
        return self.Wout // self.NWH

    def check(self):
        assert self.N <= 128
        assert self.K <= 32
        assert 32 * (self.GH - 1) + self.K <= 128
        assert self.CC * self.WB <= 512
        assert self.HWB <= 512  # mm2 span psum fits one bank
        assert self.Win == 2 * self.Wout


# ---------------------------------------------------------------- device program

def build_nc(d: Dims):
    import concourse.bacc as bacc
    import concourse.bass as bass
    import concourse.tile as tile
    from concourse import mybir

    F8 = mybir.dt.float8e3
    F16 = mybir.dt.float16
    F32 = mybir.dt.float32
    I32 = mybir.dt.int32

    d.check()
    nc = bacc.Bacc("TRN2", target_bir_lowering=False, debug=False,
                   num_devices=d.ncores)

    xpad = nc.declare_dram_parameter(
        "xpad", [2 * d.Hin, d.WPAD * d.Cin], F8, isOutput=False)
    gidx = nc.declare_dram_parameter("gidx", [d.N, d.HB], I32, isOutput=False)
    vt = nc.declare_dram_parameter("vt", [d.N, d.HB * d.KP], F16,
                                   isOutput=False)
    w2 = nc.declare_dram_parameter("w2", [d.P2, d.NG * d.Cout], F16,
                                   isOutput=False)
    y = nc.declare_dram_parameter("y", [d.HB, d.Cout, d.Wout], F16,
                                  isOutput=True)

    with tile.TileContext(nc) as tc, ExitStack() as ctx:
        const = ctx.enter_context(tc.tile_pool(name="const", bufs=1))
        xgp = ctx.enter_context(tc.tile_pool(name="xgp", bufs=2))
        z4p = ctx.enter_context(tc.tile_pool(name="z4p", bufs=2))
        zpsp = ctx.enter_context(tc.tile_pool(name="zpsp", bufs=4, space="PSUM"))
        zprp = ctx.enter_context(tc.tile_pool(name="zprp", bufs=1))
        ypsp = ctx.enter_context(tc.tile_pool(name="ypsp", bufs=4, space="PSUM"))
        ysbp = ctx.enter_context(tc.tile_pool(name="ysbp", bufs=3))

        gidx_sb = const.tile([d.N, d.HB], I32, name="gidx_sb")
        nc.sync.dma_start(out=gidx_sb[:], in_=gidx[:])
        vt_sb = const.tile([d.N, d.HB * d.KP], F16, name="vt_sb")
        nc.sync.dma_start(out=vt_sb[:], in_=vt[:])
        w2_sb = const.tile([d.P2, d.NG * d.Cout], F16, name="w2_sb")
        nc.sync.dma_start(out=w2_sb[:], in_=w2[:])

        vt_v = vt_sb.rearrange("n (h k) -> n h k", k=d.KP)
        w2_v = w2_sb.rearrange("p (g o) -> p g o", g=d.NG)

        def emit_mm2(zs4, hs_, half_):
            for j in range(d.GH):
                zv = zs4[j].rearrange("p (g w) -> p g w", g=d.NG)
                for oh in range(d.OH):
                    o0 = oh * d.OHW
                    ow = min(d.OHW, d.Cout - o0)
                    ysbt = ysbp.tile([d.OHW, d.HWB], F16, name="ysbt")
                    yps = ypsp.tile([d.OHW, d.HWB], F32, name="yps")
                    for g in range(d.NG):
                        cgg = d.CG if g < d.NG - 1 else d.CREM
                        nc.tensor.matmul(
                            out=yps[:ow, :],
                            lhsT=w2_v[:d.K * cgg, g, o0:o0 + ow],
                            rhs=zv[:d.K * cgg, g, :],
                            start=(g == 0), stop=(g == d.NG - 1),
                        )
                    if oh % 2 == 0:
                        nc.scalar.copy(out=ysbt[:ow, :], in_=yps[:ow, :])
                    else:
                        nc.vector.tensor_copy(out=ysbt[:ow, :],
                                              in_=yps[:ow, :])
                    nc.sync.dma_start(
                        out=y[hs_[j], o0:o0 + ow,
                              half_ * d.HWB:(half_ + 1) * d.HWB],
                        in_=ysbt[:ow, :])

        # mm2 for a finished span is deferred by ONE w-block: the next
        # block's mm1 fills the in-order PE queue while the span's
        # transpose DMAs drain, instead of the PE stalling on them.
        pend = None
        for grp in range(d.NGRP * d.REP):
            grp = grp % d.NGRP
            hs = [grp * d.GH + j for j in range(d.GH)]
            for wb in range(d.NWB):
                half, wl = divmod(wb, d.WBH)
                xg = []
                for j in range(d.GH):
                    xgt = xgp.tile([d.N, d.WB * d.Cin], F8, name=f"xg{j}",
                                   tag=f"xg{j}", bufs=d.XB)
                    if d.mode == "fakegather":
                        r0 = ((grp * d.NWB + wb) * d.GH + j) % \
                            (2 * d.Hin - d.N)
                        nc.sync.dma_start(
                            out=xgt[:],
                            in_=xpad[r0:r0 + d.N,
                                     wb * d.WB * d.Cin:(wb + 1) * d.WB * d.Cin])
                    else:
                        nc.gpsimd.indirect_dma_start(
                            out=xgt[:],
                            out_offset=None,
                            in_=xpad[:],
                            in_offset=bass.IndirectOffsetOnAxis(
                                ap=gidx_sb[:, hs[j]:hs[j] + 1], axis=1),
                            element_offset=wb * d.WB * d.Cin,
                        )
                    xg.append(xgt)

                if wl == 0:
                    # only 2 spans alive at once: the one being written and
                    # the previous one being transposed out
                    z4 = z4p.tile([128, d.Cin * d.HWB], F16,
                                  name=f"z4{half % 2}",
                                  tag=f"z4{half % 2}", bufs=1)
                    z4_v = z4.rearrange("p (c w) -> p c w", c=d.Cin)
                if d.stage < 2:
                    continue
                for cc in range(d.NCC):
                    c0 = cc * d.CC
                    cw = min(d.CC, d.Cin - c0)
                    zps = zpsp.tile([128, d.CC * d.WB], F32, name="zps")
                    for j in range(d.GH):
                        rhs = xg[j].rearrange("n (w c) -> n c w",
                                              c=d.Cin)[:, c0:c0 + cw, :]
                        nc.tensor.matmul(
                            out=zps[32 * j:32 * (j + 1), :cw * d.WB],
                            lhsT=vt_v[:, hs[j], :],
                            rhs=rhs,
                            start=True, stop=True,
                            tile_position=(0, 32 * j),
                        )
                    if d.stage < 3:
                        continue
                    # alternate evac between DVE and ACT to halve each
                    # engine's share of the PSUM->SBUF f32->f16 traffic
                    ev_out = z4_v[:32 * d.GH, c0:c0 + cw,
                                  wl * d.WB:(wl + 1) * d.WB]
                    ev_in = zps.rearrange("p (c w) -> p c w",
                                          c=d.CC)[:32 * d.GH, :cw, :]
                    if cc % 2 == 0:
                        nc.vector.tensor_copy(out=ev_out, in_=ev_in)
                    else:
                        nc.scalar.copy(out=ev_out, in_=ev_in)

                if d.stage >= 5 and pend is not None:
                    emit_mm2(*pend)
                    pend = None

                if d.stage < 4 or wl != d.WBH - 1:
                    continue
                # span complete: long-run transposes (HWB-wide w runs)
                # into per-span (k,c)-partition tiles.
                zs4 = []
                for j in range(d.GH):
                    zs = zprp.tile([d.P2, d.NG * d.HWB], F16,
                                   name=f"zs{j}", tag=f"zs{j}", bufs=2)
                    zv = zs.rearrange("p (g w) -> p g w", g=d.NG)
                    for g in range(d.NG):
                        cgg = d.CG if g < d.NG - 1 else d.CREM
                        # dst partitions p = k*cgg+co iterate (k, co) in the
                        # same lexicographic order as the src AP dims.
                        # NOTE: must stay on nc.sync — issuing these from
                        # nc.scalar crashed the device (NRT_EXEC_UNIT_
                        # UNRECOVERABLE), engine-issued strided SBUF->SBUF
                        # DMA is not safe here.
                        nc.sync.dma_start(
                            out=zv[:d.K * cgg, g, :],
                            in_=z4_v[32 * j:32 * j + d.K,
                                     g * d.CG:g * d.CG + cgg, :],
                        )
                    zs4.append(zs)
                if d.stage >= 5:
                    pend = (zs4, hs, half)

            if d.stage < 5:
                # keep the y-write volume, fed from a junk tile
                for j in range(d.GH):
                    for oh in range(d.OH):
                        o0 = oh * d.OHW
                        ow = min(d.OHW, d.Cout - o0)
                        ysbt = ysbp.tile([d.OHW, d.Wout], F16, name="ysbtj")
                        nc.vector.memset(ysbt[:1, :1], 0.0)
                        nc.sync.dma_start(
                            out=y[hs[j], o0:o0 + ow, :], in_=ysbt[:ow, :])
                continue
        if pend is not None:
            emit_mm2(*pend)

    nc.finalize()
    return nc


# ---------------------------------------------------------------- host side

def prep_xpad(x, d: Dims):
    import ml_dtypes
    xr = np.transpose(x[0] * np.float32(d.XSCALE), (1, 2, 0))  # [Hin,Win,Cin]
    xs = xr.reshape(d.Hin, d.Wout, d.pscale, d.Cin).transpose(2, 0, 1, 3)
    xs = np.clip(xs, -15.5, 15.5).astype(ml_dtypes.float8_e3m4)
    xpad = np.empty((2, d.Hin, d.WPAD, d.Cin), dtype=ml_dtypes.float8_e3m4)
    xpad[:, :, :d.Wout] = xs
    xpad[:, :, d.Wout:] = xs[:, :, :d.WPAD - d.Wout]
    return np.ascontiguousarray(xpad).reshape(2 * d.Hin, d.WPAD * d.Cin)


def core_h_ranges(d: Dims):
    base, rem = divmod(d.Hout, d.ncores)
    counts = [base + (1 if p < rem else 0) for p in range(d.ncores)]
    offs = np.concatenate([[0], np.cumsum(counts)])
    return [(int(offs[p]), counts[p]) for p in range(d.ncores)]


def prep_core_tables(psi_vals, idx_hi, idx_wi, d: Dims, h0, cnt):
    hg = np.minimum(h0 + np.arange(d.HB), d.Hout - 1)  # pad with a valid h
    wi = idx_wi[hg]                      # [HB, N]
    par = wi % 2
    m = wi // 2
    r = idx_hi[hg]
    flat = ((par.astype(np.int64) * d.Hin + r) * d.WPAD + m) * d.Cin
    assert flat.max() + d.Wout * d.Cin <= d.TOT
    gidx = flat.astype(np.int32).T.copy()           # [N, HB]
    vt = np.zeros((d.N, d.HB, d.KP), dtype=np.float16)
    # fold the 1/XSCALE dequant into the psi table
    vt[:, :, :d.K] = (psi_vals[:, hg, :] / np.float32(d.XSCALE)) \
        .transpose(2, 1, 0)
    return gidx, vt.reshape(d.N, d.HB * d.KP)


def prep_w2(weight, d: Dims):
    w = weight.transpose(1, 2, 0)  # [Cin, K, Cout]
    w2 = np.zeros((d.P2, d.NG, d.Cout), dtype=np.float16)
    for g in range(d.NG):
        cs = g * d.CG
        cgg = d.CG if g < d.NG - 1 else d.CREM
        # rows p = k*cgg + co
        blk = w[cs:cs + cgg].transpose(1, 0, 2).reshape(d.K * cgg, d.Cout)
        w2[:d.K * cgg, g] = blk
    return np.ascontiguousarray(w2.reshape(d.P2, d.NG * d.Cout))


_NC_CACHE = {}


def _get_nc(d: Dims):
    if d not in _NC_CACHE:
        _NC_CACHE[d] = build_nc(d)
    return _NC_CACHE[d]


def make_in_maps(x, weight, psi_vals, idx_hi, idx_wi, d: Dims):
    xpad = prep_xpad(x, d)
    w2 = prep_w2(weight, d)
    in_maps = []
    for h0, cnt in core_h_ranges(d):
        gidx, vt = prep_core_tables(psi_vals, idx_hi, idx_wi, d, h0, cnt)
        in_maps.append({"xpad": xpad, "gidx": gidx, "vt": vt, "w2": w2})
    return in_maps


def assemble_y(per_core_y, d: Dims):
    parts = [per_core_y[p][:cnt].astype(np.float32)
             for p, (h0, cnt) in enumerate(core_h_ranges(d))]
    yh = np.concatenate(parts, axis=0)          # [Hout, Cout, Wout]
    return yh.transpose(1, 0, 2)[None]          # [1, Cout, Hout, Wout]


def kernel(x, weight, psi_vals, idx_hi, idx_wi):
    from concourse.bass_utils import run_bass_kernel_spmd

    d = Dims()
    nc = _get_nc(d)
    in_maps = make_in_maps(x, weight, psi_vals, idx_hi, idx_wi, d)
    res = run_bass_kernel_spmd(nc, in_maps, list(range(d.ncores)))
    ys = [res.results[p]["y"] for p in range(d.ncores)]
    return assemble_y(ys, d).astype(x.dtype)



# revision 24
# speedup vs baseline: 1.8954x; 1.8954x over previous
"""DISCO (discrete-continuous) S2 conv encoder for Trainium2, 8-core SPMD.

Math (per output latitude h):
    z[k, c, w] = sum_n psi[k, h, n] * x[c, hi[h,n], (wi[h,n] + 2w) mod Win]
    y[o, h, w] = sum_{c,k} weight[o, c, k] * z[k, c, w]

Strategy:
  * Shard output latitudes (Hout) across the 8 cores; weight/psi tables
    replicated; each core receives the full (re-laid-out) x.
  * x is quantized host-side to float8_e3m4 (prescale XSCALE, clip to
    +-15.5); the 1/XSCALE dequant is folded into the f16 psi table.
    Mixed-dtype matmul (f16 lhsT x fp8 rhs) is legal on TRN2 and halves
    the gather DMA traffic, the dominant cost. Final rel err ~1.3e-2.
  * Host pre-lays x out as parity-split (even/odd longitude), cyclically
    padded, channel-minor rows:  xpad[par, r, w', c], w' in [0, 2*Wout-1).
    Then the (h, n) support slice {x[c, hi, wi + 2w] : w in [w0, w0+WB),
    all c} is ONE contiguous fp8 block, so the device gathers 128 of them
    (one per neighbor n) with a single indirect DMA per (h, w-block).
  * matmul1 contracts n (=128, the partition dim) with the per-h psi basis
    vT [128, K]; 4 latitudes run concurrently in separate 32-column
    tile_position groups of the PE array.
  * z is evacuated from PSUM (f32->f16, alternating DVE/ACT engines) and
    staged in NHALF w-spans; each completed span is transposed via
    SBUF->SBUF DMA (>=512B runs) into (k,c)-partition span tiles.
  * matmul2 consumes each span immediately (double-buffered per
    latitude), contracting (c,k) in NG accumulating chunks against the
    packed weight w2 [K*CG, NG, Cout]; y evacuated f16, host upcasts.
"""

import math
from contextlib import ExitStack
from dataclasses import dataclass

import numpy as np


# ---------------------------------------------------------------- dims

@dataclass(frozen=True)
class Dims:
    Cin: int = 73
    Hin: int = 721
    Win: int = 1440
    Cout: int = 256
    K: int = 9
    Hout: int = 361
    N: int = 128
    pscale: int = 2
    ncores: int = 8
    GH: int = 4     # latitudes processed concurrently (PE col-tile groups)
    WB: int = 90    # w-block size for the gather/matmul1 pipeline
    XB: int = 2     # gather buffer depth per latitude tag
    NHALF: int = 2  # z4 is staged in NHALF w-spans, transposed per span
    XSCALE: float = 2.0  # x prescale for float8e3 quantization
    EVO: int = 0    # evac AP iteration order: 0=(c,w), 1=(w,c)
    EVE: int = 0    # evac engines: 0=alternate DVE/ACT, 1=ACT only, 2=DVE only
    ZB: int = 3     # zpsp psum pool bufs (2-bank pair tiles)
    # "full" | "fakegather" | pipeline truncations for perf ablation:
    # "gather" (gathers+y only), "mm1", "evac", "tr" (everything but mm2)
    mode: str = "full"
    REP: int = 1    # repeat the whole body (device-side timing ablation)

    @property
    def stage(self):
        order = {"gather": 1, "mm1": 2, "evac": 3, "tr": 4,
                 "full": 5, "fakegather": 5}
        return order[self.mode]

    @property
    def Wout(self):
        return self.Win // self.pscale

    @property
    def WPAD(self):
        return 2 * self.Wout - 1

    @property
    def TOT(self):
        return 2 * self.Hin * self.WPAD * self.Cin

    @property
    def NWB(self):
        assert self.Wout % self.WB == 0
        return self.Wout // self.WB

    @property
    def WBH(self):  # w-blocks per z4 half-span
        assert self.NWB % self.NHALF == 0
        return self.NWB // self.NHALF

    @property
    def HWB(self):  # w width of a z4 span
        return self.WBH * self.WB

    @property
    def HB(self):  # padded per-core latitude slots
        per = math.ceil(self.Hout / self.ncores)
        return math.ceil(per / self.GH) * self.GH

    @property
    def NGRP(self):
        return self.HB // self.GH

    @property
    def KP(self):  # psi k-dim padded to a full PE column-tile group
        return 32

    @property
    def CG(self):  # channels per (c,k) partition group of z'
        return min(128 // self.K, self.Cin)

    @property
    def P2(self):
        return self.K * self.CG

    @property
    def NG(self):
        return math.ceil(self.Cin / self.CG)

    @property
    def CREM(self):  # channels in last group
        return self.Cin - self.CG * (self.NG - 1)

    @property
    def WC(self):  # w sub-chunk for matmul1 psum (<=512 f32 per bank);
        # NCW must come out even so chunks pair into 2-bank psum tiles
        for d in range(min(512 // self.Cin, self.WB), 0, -1):
            if self.WB % d == 0 and (self.WB // d) % 2 == 0:
                return d
        return 1

    @property
    def NCW(self):
        return self.WB // self.WC

    @property
    def OH(self):  # output-channel halves
        return math.ceil(self.Cout / 128)

    @property
    def OHW(self):
        return min(self.Cout, 128)

    @property
    def NWH(self):  # w-halves for matmul2 (<=512 f32 psum)
        return math.ceil(self.Wout / 512)

    @property
    def WHS(self):
        assert self.Wout % self.NWH == 0
        return self.Wout // self.NWH

    def check(self):
        assert self.N <= 128
        assert self.K <= 32
        assert 32 * (self.GH - 1) + self.K <= 128
        assert self.WC * self.Cin <= 512
        assert self.WB % self.WC == 0
        assert self.HWB <= 512  # mm2 span psum fits one bank
        assert self.Win == 2 * self.Wout


# ---------------------------------------------------------------- device program

def build_nc(d: Dims):
    import concourse.bacc as bacc
    import concourse.bass as bass
    import concourse.tile as tile
    from concourse import mybir

    F8 = mybir.dt.float8e3
    F16 = mybir.dt.float16
    F32 = mybir.dt.float32
    I32 = mybir.dt.int32

    d.check()
    nc = bacc.Bacc("TRN2", target_bir_lowering=False, debug=False,
                   num_devices=d.ncores)

    xpad = nc.declare_dram_parameter(
        "xpad", [2 * d.Hin, d.WPAD * d.Cin], F8, isOutput=False)
    gidx = nc.declare_dram_parameter("gidx", [d.N, d.HB], I32, isOutput=False)
    vt = nc.declare_dram_parameter("vt", [d.N, d.HB * d.KP], F16,
                                   isOutput=False)
    w2 = nc.declare_dram_parameter("w2", [d.P2, d.NG * d.Cout], F16,
                                   isOutput=False)
    y = nc.declare_dram_parameter("y", [d.HB, d.Cout, d.Wout], F16,
                                  isOutput=True)

    with tile.TileContext(nc) as tc, ExitStack() as ctx:
        const = ctx.enter_context(tc.tile_pool(name="const", bufs=1))
        xgp = ctx.enter_context(tc.tile_pool(name="xgp", bufs=2))
        z4p = ctx.enter_context(tc.tile_pool(name="z4p", bufs=2))
        zpsp = ctx.enter_context(tc.tile_pool(name="zpsp", bufs=d.ZB,
                                              space="PSUM"))
        zprp = ctx.enter_context(tc.tile_pool(name="zprp", bufs=1))
        ypsp = ctx.enter_context(tc.tile_pool(name="ypsp", bufs=2, space="PSUM"))
        ysbp = ctx.enter_context(tc.tile_pool(name="ysbp", bufs=5))

        gidx_sb = const.tile([d.N, d.HB], I32, name="gidx_sb")
        nc.sync.dma_start(out=gidx_sb[:], in_=gidx[:])
        vt_sb = const.tile([d.N, d.HB * d.KP], F16, name="vt_sb")
        nc.sync.dma_start(out=vt_sb[:], in_=vt[:])
        w2_sb = const.tile([d.P2, d.NG * d.Cout], F16, name="w2_sb")
        nc.sync.dma_start(out=w2_sb[:], in_=w2[:])

        vt_v = vt_sb.rearrange("n (h k) -> n h k", k=d.KP)
        w2_v = w2_sb.rearrange("p (g o) -> p g o", g=d.NG)

        def emit_mm2(zs4, hs_, half_):
            zvs = [z.rearrange("p (g w) -> p g w", g=d.NG) for z in zs4]
            ysbts = [ysbp.tile([d.OHW, d.OH * d.HWB], F16, name="ysbt")
                     for _ in range(d.GH)]
            for j in range(d.GH):
                for oh in range(d.OH):
                    o0 = oh * d.OHW
                    ow = min(d.OHW, d.Cout - o0)
                    yps = ypsp.tile([d.OHW, d.HWB], F32, name="yps")
                    for g in range(d.NG):
                        cgg = d.CG if g < d.NG - 1 else d.CREM
                        nc.tensor.matmul(
                            out=yps[:ow, :],
                            lhsT=w2_v[:d.K * cgg, g, o0:o0 + ow],
                            rhs=zvs[j][:d.K * cgg, g, :],
                            start=(g == 0), stop=(g == d.NG - 1),
                        )
                    ev = ysbts[j][:ow, oh * d.HWB:(oh + 1) * d.HWB]
                    if (oh + j) % 2 == 0:
                        nc.scalar.copy(out=ev, in_=yps[:ow, :])
                    else:
                        nc.vector.tensor_copy(out=ev, in_=yps[:ow, :])
            for j in range(d.GH):
                # one write per latitude: [op, (oh w)] -> y[h, (oh op), w]
                out_ap = y[hs_[j], :,
                           half_ * d.HWB:(half_ + 1) * d.HWB] \
                    .rearrange("(oh op) w -> op oh w", oh=d.OH)
                nc.sync.dma_start(
                    out=out_ap,
                    in_=ysbts[j].rearrange("p (oh w) -> p oh w", oh=d.OH))

        # mm2 for a finished span is deferred by ONE w-block: the next
        # block's mm1 fills the in-order PE queue while the span's
        # transpose DMAs drain, instead of the PE stalling on them.
        pend = None
        for grp in range(d.NGRP * d.REP):
            grp = grp % d.NGRP
            hs = [grp * d.GH + j for j in range(d.GH)]
            for wb in range(d.NWB):
                half, wl = divmod(wb, d.WBH)
                xg = []
                for j in range(d.GH):
                    xgt = xgp.tile([d.N, d.WB * d.Cin], F8, name=f"xg{j}",
                                   tag=f"xg{j}", bufs=d.XB)
                    if d.mode == "fakegather":
                        r0 = ((grp * d.NWB + wb) * d.GH + j) % \
                            (2 * d.Hin - d.N)
                        nc.sync.dma_start(
                            out=xgt[:],
                            in_=xpad[r0:r0 + d.N,
                                     wb * d.WB * d.Cin:(wb + 1) * d.WB * d.Cin])
                    else:
                        nc.gpsimd.indirect_dma_start(
                            out=xgt[:],
                            out_offset=None,
                            in_=xpad[:],
                            in_offset=bass.IndirectOffsetOnAxis(
                                ap=gidx_sb[:, hs[j]:hs[j] + 1], axis=1),
                            element_offset=wb * d.WB * d.Cin,
                        )
                    xg.append(xgt)

                if wl == 0:
                    # only 2 spans alive at once: the one being written and
                    # the previous one being transposed out.
                    # channel SLOT order is interleaved: natural channel
                    # c = g*CG+co (g < NG-1) lives at slot co*(NG-1)+g, so
                    # the per-(k,co) transpose reads a contiguous (g,w) run;
                    # CREM channels keep natural slots [CG*(NG-1), Cin).
                    z4 = z4p.tile([128, d.Cin * d.HWB], F16,
                                  name=f"z4{half % 2}",
                                  tag=f"z4{half % 2}", bufs=1)
                    z4_v = z4.rearrange("p (c w) -> p c w", c=d.Cin)
                if d.stage < 2:
                    continue
                for cp in range(d.NCW // 2):
                    # two w-chunks share one 2-bank psum tile so the evac
                    # is a single bigger op (engine queues are 8-deep
                    # strict FIFO; fewer, larger cross-engine handoffs)
                    zps = zpsp.tile([128, 1024], F32, name="zps")
                    for sub in range(2):
                        w0 = (cp * 2 + sub) * d.WC
                        for j in range(d.GH):
                            # contiguous (w-major) rhs slice: the PE
                            # streams strided APs ~5x slower, and
                            # back-to-back issue lets the 4 col-tile
                            # groups overlap
                            rhs = xg[j][:, w0 * d.Cin:(w0 + d.WC) * d.Cin]
                            nc.tensor.matmul(
                                out=zps[32 * j:32 * (j + 1),
                                        512 * sub:512 * sub + d.WC * d.Cin],
                                lhsT=vt_v[:, hs[j], :],
                                rhs=rhs,
                                start=True, stop=True,
                                tile_position=(0, 32 * j),
                            )
                    if d.stage < 3:
                        continue
                    # alternate evac between DVE and ACT to halve each
                    # engine's share of the PSUM->SBUF f32->f16 traffic.
                    # psum columns are (sub, w, s) where s is already the
                    # z4 slot order (host permutes x channels), so one
                    # 3-dim strided copy per pair evacuates everything.
                    base = wl * d.WB + cp * 2 * d.WC
                    ev_out = z4_v[:, :, base:base + 2 * d.WC]
                    ev_in = zps.rearrange("p (h x) -> p h x", h=2) \
                        [:, :, :d.WC * d.Cin] \
                        .rearrange("p h (w c) -> p c h w", w=d.WC)
                    use_dve = (d.EVE == 2) or (d.EVE == 0 and cp % 2 == 0)
                    if use_dve:
                        nc.vector.tensor_copy(out=ev_out, in_=ev_in)
                    else:
                        nc.scalar.copy(out=ev_out, in_=ev_in)

                if d.stage >= 5 and pend is not None:
                    emit_mm2(*pend)
                    pend = None

                if d.stage < 4 or wl != d.WBH - 1:
                    continue
                # span complete: long-run transposes (HWB-wide w runs)
                # into per-span (k,c)-partition tiles.
                zs4 = []
                nmain = d.NG - 1  # groups with cgg == CG
                for j in range(d.GH):
                    zs = zprp.tile([d.P2, d.NG * d.HWB], F16,
                                   name=f"zs{j}", tag=f"zs{j}", bufs=2)
                    zv = zs.rearrange("p (g w) -> p g w", g=d.NG)
                    # dst partitions p = k*cgg+co iterate (k, co) in the
                    # same lexicographic order as the src AP dims; all
                    # full-CG groups merged into one op (g becomes a free
                    # dim on both sides), CREM tail separate.
                    # NOTE: must stay on nc.sync — issuing these from
                    # nc.scalar crashed the device (NRT_EXEC_UNIT_
                    # UNRECOVERABLE), engine-issued strided SBUF->SBUF
                    # DMA is not safe here.
                    nc.sync.dma_start(
                        out=zs[:d.K * d.CG, :nmain * d.HWB],
                        in_=z4_v[32 * j:32 * j + d.K, :nmain * d.CG, :]
                        .rearrange("k (co g) w -> k co (g w)", co=d.CG),
                    )
                    nc.sync.dma_start(
                        out=zv[:d.K * d.CREM, nmain, :],
                        in_=z4_v[32 * j:32 * j + d.K,
                                 nmain * d.CG:d.Cin, :],
                    )
                    zs4.append(zs)
                if d.stage >= 5:
                    pend = (zs4, hs, half)

            if d.stage < 5:
                # keep the y-write volume, fed from a junk tile
                for j in range(d.GH):
                    for oh in range(d.OH):
                        o0 = oh * d.OHW
                        ow = min(d.OHW, d.Cout - o0)
                        ysbt = ysbp.tile([d.OHW, d.Wout], F16, name="ysbtj")
                        nc.vector.memset(ysbt[:1, :1], 0.0)
                        nc.sync.dma_start(
                            out=y[hs[j], o0:o0 + ow, :], in_=ysbt[:ow, :])
                continue
        if pend is not None:
            emit_mm2(*pend)

    nc.finalize()
    return nc


# ---------------------------------------------------------------- host side

def chan_perm(d: Dims):
    """z4/psum slot s -> natural channel. Slot order co*(NG-1)+g makes the
    per-(k,co) transpose read a contiguous (g,w) run; CREM slots natural."""
    nmain = d.NG - 1
    perm = np.arange(d.Cin)
    main = d.CG * nmain
    s = np.arange(main)
    perm[:main] = (s % nmain) * d.CG + s // nmain
    return perm


def prep_xpad(x, d: Dims):
    import ml_dtypes
    xr = np.transpose(x[0] * np.float32(d.XSCALE), (1, 2, 0))  # [Hin,Win,Cin]
    xr = xr[:, :, chan_perm(d)]
    xs = xr.reshape(d.Hin, d.Wout, d.pscale, d.Cin).transpose(2, 0, 1, 3)
    xs = np.clip(xs, -15.5, 15.5).astype(ml_dtypes.float8_e3m4)
    xpad = np.empty((2, d.Hin, d.WPAD, d.Cin), dtype=ml_dtypes.float8_e3m4)
    xpad[:, :, :d.Wout] = xs
    xpad[:, :, d.Wout:] = xs[:, :, :d.WPAD - d.Wout]
    return np.ascontiguousarray(xpad).reshape(2 * d.Hin, d.WPAD * d.Cin)


def core_h_ranges(d: Dims):
    base, rem = divmod(d.Hout, d.ncores)
    counts = [base + (1 if p < rem else 0) for p in range(d.ncores)]
    offs = np.concatenate([[0], np.cumsum(counts)])
    return [(int(offs[p]), counts[p]) for p in range(d.ncores)]


def prep_core_tables(psi_vals, idx_hi, idx_wi, d: Dims, h0, cnt):
    hg = np.minimum(h0 + np.arange(d.HB), d.Hout - 1)  # pad with a valid h
    wi = idx_wi[hg]                      # [HB, N]
    par = wi % 2
    m = wi // 2
    r = idx_hi[hg]
    flat = ((par.astype(np.int64) * d.Hin + r) * d.WPAD + m) * d.Cin
    assert flat.max() + d.Wout * d.Cin <= d.TOT
    gidx = flat.astype(np.int32).T.copy()           # [N, HB]
    vt = np.zeros((d.N, d.HB, d.KP), dtype=np.float16)
    # fold the 1/XSCALE dequant into the psi table
    vt[:, :, :d.K] = (psi_vals[:, hg, :] / np.float32(d.XSCALE)) \
        .transpose(2, 1, 0)
    return gidx, vt.reshape(d.N, d.HB * d.KP)


def prep_w2(weight, d: Dims):
    w = weight.transpose(1, 2, 0)  # [Cin, K, Cout]
    w2 = np.zeros((d.P2, d.NG, d.Cout), dtype=np.float16)
    for g in range(d.NG):
        cs = g * d.CG
        cgg = d.CG if g < d.NG - 1 else d.CREM
        # rows p = k*cgg + co
        blk = w[cs:cs + cgg].transpose(1, 0, 2).reshape(d.K * cgg, d.Cout)
        w2[:d.K * cgg, g] = blk
    return np.ascontiguousarray(w2.reshape(d.P2, d.NG * d.Cout))


_NC_CACHE = {}


def _get_nc(d: Dims):
    if d not in _NC_CACHE:
        _NC_CACHE[d] = build_nc(d)
    return _NC_CACHE[d]


def make_in_maps(x, weight, psi_vals, idx_hi, idx_wi, d: Dims):
    xpad = prep_xpad(x, d)
    w2 = prep_w2(weight, d)
    in_maps = []
    for h0, cnt in core_h_ranges(d):
        gidx, vt = prep_core_tables(psi_vals, idx_hi, idx_wi, d, h0, cnt)
        in_maps.append({"xpad": xpad, "gidx": gidx, "vt": vt, "w2": w2})
    return in_maps


def assemble_y(per_core_y, d: Dims):
    parts = [per_core_y[p][:cnt].astype(np.float32)
             for p, (h0, cnt) in enumerate(core_h_ranges(d))]
    yh = np.concatenate(parts, axis=0)          # [Hout, Cout, Wout]
    return yh.transpose(1, 0, 2)[None]          # [1, Cout, Hout, Wout]


def kernel(x, weight, psi_vals, idx_hi, idx_wi):
    from concourse.bass_utils import run_bass_kernel_spmd

    d = Dims()
    nc = _get_nc(d)
    in_maps = make_in_maps(x, weight, psi_vals, idx_hi, idx_wi, d)
    res = run_bass_kernel_spmd(nc, in_maps, list(range(d.ncores)))
    ys = [res.results[p]["y"] for p in range(d.ncores)]
    return assemble_y(ys, d).astype(x.dtype)



# revision 30
# speedup vs baseline: 2.1724x; 1.1462x over previous
"""DISCO (discrete-continuous) S2 conv encoder for Trainium2, 8-core SPMD.

Math (per output latitude h):
    z[k, c, w] = sum_n psi[k, h, n] * x[c, hi[h,n], (wi[h,n] + 2w) mod Win]
    y[o, h, w] = sum_{c,k} weight[o, c, k] * z[k, c, w]

Strategy:
  * Shard output latitudes (Hout) across the 8 cores; weight/psi tables
    replicated; each core receives the full (re-laid-out) x.
  * x is quantized host-side to float8_e3m4 (prescale XSCALE, clip to
    +-15.5); the 1/XSCALE dequant is folded into the f16 psi table.
    Mixed-dtype matmul (f16 lhsT x fp8 rhs) is legal on TRN2 and halves
    the gather DMA traffic, the dominant cost. Final rel err ~1.3e-2.
  * Host pre-lays x out as parity-split (even/odd longitude), cyclically
    padded, channel-minor rows:  xpad[par, r, w', c], w' in [0, 2*Wout-1).
    Then the (h, n) support slice {x[c, hi, wi + 2w] : w in [w0, w0+WB),
    all c} is ONE contiguous fp8 block, so the device gathers 128 of them
    (one per neighbor n) with a single indirect DMA per (h, w-block).
  * matmul1 contracts n (=128, the partition dim) with the per-h psi basis
    vT [128, K]; 4 latitudes run concurrently in separate 32-column
    tile_position groups of the PE array.
  * z is evacuated from PSUM (f32->f16, alternating DVE/ACT engines) and
    staged in NHALF w-spans; each completed span is transposed via
    SBUF->SBUF DMA (>=512B runs) into (k,c)-partition span tiles.
  * matmul2 consumes each span immediately (double-buffered per
    latitude), contracting (c,k) in NG accumulating chunks against the
    packed weight w2 [K*CG, NG, Cout]; y evacuated f16, host upcasts.
"""

import math
from contextlib import ExitStack
from dataclasses import dataclass

import numpy as np


# ---------------------------------------------------------------- dims

@dataclass(frozen=True)
class Dims:
    Cin: int = 73
    Hin: int = 721
    Win: int = 1440
    Cout: int = 256
    K: int = 9
    Hout: int = 361
    N: int = 128
    pscale: int = 2
    ncores: int = 8
    GH: int = 4     # latitudes processed concurrently (PE col-tile groups)
    WB: int = 90    # w-block size for the gather/matmul1 pipeline
    XB: int = 2     # gather buffer depth per latitude tag
    NHALF: int = 2  # z4 is staged in NHALF w-spans, transposed per span
    XSCALE: float = 2.0  # x prescale for float8e3 quantization
    EVO: int = 0    # evac AP iteration order: 0=(c,w), 1=(w,c)
    EVE: int = 0    # evac engines: 0=alternate DVE/ACT, 1=ACT only, 2=DVE only
    ZB: int = 3     # zpsp psum pool bufs (2-bank pair tiles)
    # "full" | "fakegather" | pipeline truncations for perf ablation:
    # "gather" (gathers+y only), "mm1", "evac", "tr" (everything but mm2)
    mode: str = "full"
    REP: int = 1    # repeat the whole body (device-side timing ablation)

    @property
    def stage(self):
        order = {"gather": 1, "mm1": 2, "evac": 3, "tr": 4,
                 "full": 5, "fakegather": 5}
        return order[self.mode]

    @property
    def Wout(self):
        return self.Win // self.pscale

    @property
    def WPAD(self):
        return 2 * self.Wout - 1

    @property
    def TOT(self):
        return 2 * self.Hin * self.WPAD * self.Cin

    @property
    def NWB(self):
        assert self.Wout % self.WB == 0
        return self.Wout // self.WB

    @property
    def WBH(self):  # w-blocks per z4 half-span
        assert self.NWB % self.NHALF == 0
        return self.NWB // self.NHALF

    @property
    def HWB(self):  # w width of a z4 span
        return self.WBH * self.WB

    @property
    def HB(self):  # padded per-core latitude slots
        per = math.ceil(self.Hout / self.ncores)
        return math.ceil(per / self.GH) * self.GH

    @property
    def NGRP(self):
        return self.HB // self.GH

    @property
    def KP(self):  # psi k-dim padded to a full PE column-tile group
        return 32

    @property
    def CG(self):  # channels per (c,k) partition group of z'
        return min(128 // self.K, self.Cin)

    @property
    def P2(self):
        return self.K * self.CG

    @property
    def NG(self):
        return math.ceil(self.Cin / self.CG)

    @property
    def CREM(self):  # channels in last group
        return self.Cin - self.CG * (self.NG - 1)

    @property
    def WC(self):  # w sub-chunk for matmul1 psum (<=512 f32 per bank);
        # NCW must come out even so chunks pair into 2-bank psum tiles
        for d in range(min(512 // self.Cin, self.WB), 0, -1):
            if self.WB % d == 0 and (self.WB // d) % 2 == 0:
                return d
        return 1

    @property
    def NCW(self):
        return self.WB // self.WC

    @property
    def OH(self):  # output-channel halves
        return math.ceil(self.Cout / 128)

    @property
    def OHW(self):
        return min(self.Cout, 128)

    @property
    def NWH(self):  # w-halves for matmul2 (<=512 f32 psum)
        return math.ceil(self.Wout / 512)

    @property
    def WHS(self):
        assert self.Wout % self.NWH == 0
        return self.Wout // self.NWH

    def check(self):
        assert self.N <= 128
        assert self.K <= 32
        assert 32 * (self.GH - 1) + self.K <= 128
        assert self.WC * self.Cin <= 512
        assert self.WB % self.WC == 0
        assert self.HWB <= 512  # mm2 span psum fits one bank
        assert self.Win == 2 * self.Wout


# ---------------------------------------------------------------- device program

def build_nc(d: Dims):
    import concourse.bacc as bacc
    import concourse.bass as bass
    import concourse.tile as tile
    from concourse import mybir

    F8 = mybir.dt.float8e3
    F16 = mybir.dt.float16
    F32 = mybir.dt.float32
    I32 = mybir.dt.int32

    d.check()
    nc = bacc.Bacc("TRN2", target_bir_lowering=False, debug=False,
                   num_devices=d.ncores)

    xpad = nc.declare_dram_parameter(
        "xpad", [2 * d.Hin, d.WPAD * d.Cin], F8, isOutput=False)
    gidx = nc.declare_dram_parameter("gidx", [d.N, d.HB], I32, isOutput=False)
    vt = nc.declare_dram_parameter("vt", [d.N, d.HB * d.KP], F16,
                                   isOutput=False)
    w2 = nc.declare_dram_parameter("w2", [d.P2, d.NG * d.Cout], F16,
                                   isOutput=False)
    y = nc.declare_dram_parameter("y", [d.HB, d.Cout, d.Wout], F16,
                                  isOutput=True)

    with tile.TileContext(nc) as tc, ExitStack() as ctx:
        const = ctx.enter_context(tc.tile_pool(name="const", bufs=1))
        xgp = ctx.enter_context(tc.tile_pool(name="xgp", bufs=2))
        z4p = ctx.enter_context(tc.tile_pool(name="z4p", bufs=2))
        zpsp = ctx.enter_context(tc.tile_pool(name="zpsp", bufs=d.ZB,
                                              space="PSUM"))
        zprp = ctx.enter_context(tc.tile_pool(name="zprp", bufs=1))
        ypsp = ctx.enter_context(tc.tile_pool(name="ypsp", bufs=2, space="PSUM"))
        ysbp = ctx.enter_context(tc.tile_pool(name="ysbp", bufs=5))

        gidx_sb = const.tile([d.N, d.HB], I32, name="gidx_sb")
        nc.sync.dma_start(out=gidx_sb[:], in_=gidx[:])
        vt_sb = const.tile([d.N, d.HB * d.KP], F16, name="vt_sb")
        nc.sync.dma_start(out=vt_sb[:], in_=vt[:])
        w2_sb = const.tile([d.P2, d.NG * d.Cout], F16, name="w2_sb")
        nc.sync.dma_start(out=w2_sb[:], in_=w2[:])

        vt_v = vt_sb.rearrange("n (h k) -> n h k", k=d.KP)
        w2_v = w2_sb.rearrange("p (g o) -> p g o", g=d.NG)

        def emit_mm2(zs4, hs_, half_):
            zvs = [z.rearrange("p (g w) -> p g w", g=d.NG) for z in zs4]
            ysbts = [ysbp.tile([d.OHW, d.OH * d.HWB], F16, name="ysbt")
                     for _ in range(d.GH)]
            for j in range(d.GH):
                for oh in range(d.OH):
                    o0 = oh * d.OHW
                    ow = min(d.OHW, d.Cout - o0)
                    yps = ypsp.tile([d.OHW, d.HWB], F32, name="yps")
                    for g in range(d.NG):
                        cgg = d.CG if g < d.NG - 1 else d.CREM
                        nc.tensor.matmul(
                            out=yps[:ow, :],
                            lhsT=w2_v[:d.K * cgg, g, o0:o0 + ow],
                            rhs=zvs[j][:d.K * cgg, g, :],
                            start=(g == 0), stop=(g == d.NG - 1),
                        )
                    ev = ysbts[j][:ow, oh * d.HWB:(oh + 1) * d.HWB]
                    if (oh + j) % 2 == 0:
                        nc.scalar.copy(out=ev, in_=yps[:ow, :])
                    else:
                        nc.vector.tensor_copy(out=ev, in_=yps[:ow, :])
            for j in range(d.GH):
                # one write per latitude: [op, (oh w)] -> y[h, (oh op), w]
                out_ap = y[hs_[j], :,
                           half_ * d.HWB:(half_ + 1) * d.HWB] \
                    .rearrange("(oh op) w -> op oh w", oh=d.OH)
                nc.sync.dma_start(
                    out=out_ap,
                    in_=ysbts[j].rearrange("p (oh w) -> p oh w", oh=d.OH))

        # mm2 for a finished span is deferred by ONE w-block: the next
        # block's mm1 fills the in-order PE queue while the span's
        # transpose DMAs drain, instead of the PE stalling on them.
        pend = None
        pend_ev = None
        for grp in range(d.NGRP * d.REP):
            grp = grp % d.NGRP
            hs = [grp * d.GH + j for j in range(d.GH)]
            for wb in range(d.NWB):
                half, wl = divmod(wb, d.WBH)
                xg = []
                for j in range(d.GH):
                    xgt = xgp.tile([d.N, d.WB * d.Cin], F8, name=f"xg{j}",
                                   tag=f"xg{j}", bufs=d.XB)
                    if d.mode == "fakegather":
                        r0 = ((grp * d.NWB + wb) * d.GH + j) % \
                            (2 * d.Hin - d.N)
                        nc.sync.dma_start(
                            out=xgt[:],
                            in_=xpad[r0:r0 + d.N,
                                     wb * d.WB * d.Cin:(wb + 1) * d.WB * d.Cin])
                    else:
                        nc.gpsimd.indirect_dma_start(
                            out=xgt[:],
                            out_offset=None,
                            in_=xpad[:],
                            in_offset=bass.IndirectOffsetOnAxis(
                                ap=gidx_sb[:, hs[j]:hs[j] + 1], axis=1),
                            element_offset=wb * d.WB * d.Cin,
                        )
                    xg.append(xgt)

                if wl == 0:
                    # only 2 spans alive at once: the one being written and
                    # the previous one being transposed out.
                    # channel SLOT order is interleaved: natural channel
                    # c = g*CG+co (g < NG-1) lives at slot co*(NG-1)+g, so
                    # the per-(k,co) transpose reads a contiguous (g,w) run;
                    # CREM channels keep natural slots [CG*(NG-1), Cin).
                    z4 = z4p.tile([128, d.Cin * d.HWB], F16,
                                  name=f"z4{half % 2}",
                                  tag=f"z4{half % 2}", bufs=1)
                    z4_v = z4.rearrange("p (c w) -> p c w", c=d.Cin)
                if d.stage < 2:
                    continue
                for cp in range(d.NCW // 2):
                    if pend_ev is not None:
                        # evac deferred by one pair: the next pair's MMs
                        # enter the PE queue first, so the engines drain
                        # psum while the PE streams ahead
                        o_, i_, pdve = pend_ev
                        if pdve:
                            nc.vector.tensor_copy(out=o_, in_=i_)
                        else:
                            nc.scalar.copy(out=o_, in_=i_)
                        pend_ev = None
                    # two w-chunks share one 2-bank psum tile so the evac
                    # is a single bigger op (engine queues are 8-deep
                    # strict FIFO; fewer, larger cross-engine handoffs)
                    zps = zpsp.tile([128, 1024], F32, name="zps")
                    for sub in range(2):
                        w0 = (cp * 2 + sub) * d.WC
                        for j in range(d.GH):
                            # contiguous (w-major) rhs slice: the PE
                            # streams strided APs ~5x slower, and
                            # back-to-back issue lets the 4 col-tile
                            # groups overlap
                            rhs = xg[j][:, w0 * d.Cin:(w0 + d.WC) * d.Cin]
                            nc.tensor.matmul(
                                out=zps[32 * j:32 * (j + 1),
                                        512 * sub:512 * sub + d.WC * d.Cin],
                                lhsT=vt_v[:, hs[j], :],
                                rhs=rhs,
                                start=True, stop=True,
                                tile_position=(0, 32 * j),
                            )
                    if d.stage < 3:
                        continue
                    # alternate evac between DVE and ACT to halve each
                    # engine's share of the PSUM->SBUF f32->f16 traffic.
                    # psum columns are (sub, w, s) where s is already the
                    # z4 slot order (host permutes x channels), so one
                    # 3-dim strided copy per pair evacuates everything.
                    base = wl * d.WB + cp * 2 * d.WC
                    ev_out = z4_v[:, :, base:base + 2 * d.WC]
                    ev_in = zps.rearrange("p (h x) -> p h x", h=2) \
                        [:, :, :d.WC * d.Cin] \
                        .rearrange("p h (w c) -> p c h w", w=d.WC)
                    use_dve = (d.EVE == 2) or (d.EVE == 0 and cp % 2 == 0)
                    pend_ev = (ev_out, ev_in, use_dve)

                if d.stage >= 3 and pend_ev is not None:
                    # flush before span-end transposes / next wb
                    o_, i_, pdve = pend_ev
                    if pdve:
                        nc.vector.tensor_copy(out=o_, in_=i_)
                    else:
                        nc.scalar.copy(out=o_, in_=i_)
                    pend_ev = None

                if d.stage >= 5 and pend is not None:
                    emit_mm2(*pend)
                    pend = None

                if d.stage < 4 or wl != d.WBH - 1:
                    continue
                # span complete: long-run transposes (HWB-wide w runs)
                # into per-span (k,c)-partition tiles.
                zs4 = []
                nmain = d.NG - 1  # groups with cgg == CG
                for j in range(d.GH):
                    zs = zprp.tile([d.P2, d.NG * d.HWB], F16,
                                   name=f"zs{j}", tag=f"zs{j}", bufs=2)
                    zv = zs.rearrange("p (g w) -> p g w", g=d.NG)
                    # dst partitions p = k*cgg+co iterate (k, co) in the
                    # same lexicographic order as the src AP dims; all
                    # full-CG groups merged into one op (g becomes a free
                    # dim on both sides), CREM tail separate.
                    # NOTE: must stay on nc.sync — issuing these from
                    # nc.scalar crashed the device (NRT_EXEC_UNIT_
                    # UNRECOVERABLE), engine-issued strided SBUF->SBUF
                    # DMA is not safe here.
                    # split the big span transposes across the SP HWDGE
                    # ring and the gpsimd SWDGE path to probe/relieve
                    # per-ring serialization
                    tr_eng = nc.gpsimd
                    tr_eng.dma_start(
                        out=zs[:d.K * d.CG, :nmain * d.HWB],
                        in_=z4_v[32 * j:32 * j + d.K, :nmain * d.CG, :]
                        .rearrange("k (co g) w -> k co (g w)", co=d.CG),
                    )
                    nc.sync.dma_start(
                        out=zv[:d.K * d.CREM, nmain, :],
                        in_=z4_v[32 * j:32 * j + d.K,
                                 nmain * d.CG:d.Cin, :],
                    )
                    zs4.append(zs)
                if d.stage >= 5:
                    pend = (zs4, hs, half)

            if d.stage < 5:
                # keep the y-write volume, fed from a junk tile
                for j in range(d.GH):
                    for oh in range(d.OH):
                        o0 = oh * d.OHW
                        ow = min(d.OHW, d.Cout - o0)
                        ysbt = ysbp.tile([d.OHW, d.Wout], F16, name="ysbtj")
                        nc.vector.memset(ysbt[:1, :1], 0.0)
                        nc.sync.dma_start(
                            out=y[hs[j], o0:o0 + ow, :], in_=ysbt[:ow, :])
                continue
        if pend is not None:
            emit_mm2(*pend)

    nc.finalize()
    return nc


# ---------------------------------------------------------------- host side

def chan_perm(d: Dims):
    """z4/psum slot s -> natural channel. Slot order co*(NG-1)+g makes the
    per-(k,co) transpose read a contiguous (g,w) run; CREM slots natural."""
    nmain = d.NG - 1
    perm = np.arange(d.Cin)
    main = d.CG * nmain
    s = np.arange(main)
    perm[:main] = (s % nmain) * d.CG + s // nmain
    return perm


def prep_xpad(x, d: Dims):
    import ml_dtypes
    xr = np.transpose(x[0] * np.float32(d.XSCALE), (1, 2, 0))  # [Hin,Win,Cin]
    xr = xr[:, :, chan_perm(d)]
    xs = xr.reshape(d.Hin, d.Wout, d.pscale, d.Cin).transpose(2, 0, 1, 3)
    xs = np.clip(xs, -15.5, 15.5).astype(ml_dtypes.float8_e3m4)
    xpad = np.empty((2, d.Hin, d.WPAD, d.Cin), dtype=ml_dtypes.float8_e3m4)
    xpad[:, :, :d.Wout] = xs
    xpad[:, :, d.Wout:] = xs[:, :, :d.WPAD - d.Wout]
    return np.ascontiguousarray(xpad).reshape(2 * d.Hin, d.WPAD * d.Cin)


def core_h_ranges(d: Dims):
    base, rem = divmod(d.Hout, d.ncores)
    counts = [base + (1 if p < rem else 0) for p in range(d.ncores)]
    offs = np.concatenate([[0], np.cumsum(counts)])
    return [(int(offs[p]), counts[p]) for p in range(d.ncores)]


def prep_core_tables(psi_vals, idx_hi, idx_wi, d: Dims, h0, cnt):
    hg = np.minimum(h0 + np.arange(d.HB), d.Hout - 1)  # pad with a valid h
    wi = idx_wi[hg]                      # [HB, N]
    par = wi % 2
    m = wi // 2
    r = idx_hi[hg]
    flat = ((par.astype(np.int64) * d.Hin + r) * d.WPAD + m) * d.Cin
    assert flat.max() + d.Wout * d.Cin <= d.TOT
    gidx = flat.astype(np.int32).T.copy()           # [N, HB]
    vt = np.zeros((d.N, d.HB, d.KP), dtype=np.float16)
    # fold the 1/XSCALE dequant into the psi table
    vt[:, :, :d.K] = (psi_vals[:, hg, :] / np.float32(d.XSCALE)) \
        .transpose(2, 1, 0)
    return gidx, vt.reshape(d.N, d.HB * d.KP)


def prep_w2(weight, d: Dims):
    w = weight.transpose(1, 2, 0)  # [Cin, K, Cout]
    w2 = np.zeros((d.P2, d.NG, d.Cout), dtype=np.float16)
    for g in range(d.NG):
        cs = g * d.CG
        cgg = d.CG if g < d.NG - 1 else d.CREM
        # rows p = k*cgg + co
        blk = w[cs:cs + cgg].transpose(1, 0, 2).reshape(d.K * cgg, d.Cout)
        w2[:d.K * cgg, g] = blk
    return np.ascontiguousarray(w2.reshape(d.P2, d.NG * d.Cout))


_NC_CACHE = {}


def _get_nc(d: Dims):
    if d not in _NC_CACHE:
        _NC_CACHE[d] = build_nc(d)
    return _NC_CACHE[d]


def make_in_maps(x, weight, psi_vals, idx_hi, idx_wi, d: Dims):
    xpad = prep_xpad(x, d)
    w2 = prep_w2(weight, d)
    in_maps = []
    for h0, cnt in core_h_ranges(d):
        gidx, vt = prep_core_tables(psi_vals, idx_hi, idx_wi, d, h0, cnt)
        in_maps.append({"xpad": xpad, "gidx": gidx, "vt": vt, "w2": w2})
    return in_maps


def assemble_y(per_core_y, d: Dims):
    parts = [per_core_y[p][:cnt].astype(np.float32)
             for p, (h0, cnt) in enumerate(core_h_ranges(d))]
    yh = np.concatenate(parts, axis=0)          # [Hout, Cout, Wout]
    return yh.transpose(1, 0, 2)[None]          # [1, Cout, Hout, Wout]


def kernel(x, weight, psi_vals, idx_hi, idx_wi):
    from concourse.bass_utils import run_bass_kernel_spmd

    d = Dims()
    nc = _get_nc(d)
    in_maps = make_in_maps(x, weight, psi_vals, idx_hi, idx_wi, d)
    res = run_bass_kernel_spmd(nc, in_maps, list(range(d.ncores)))
    ys = [res.results[p]["y"] for p in range(d.ncores)]
    return assemble_y(ys, d).astype(x.dtype)

